# revision 25
# baseline (speedup 1.0000x reference)
# Trainium2 Bass kernel for ClassAttn (single class-token query attention).
#
# Math (per batch b):
#   q   = x[b,0] @ Wq * scale                       [CR]
#   logits[h,n] = sum_d q[h,d] * (x[b] @ Wk)[n,h,d]
#               = sum_c x[b,n,c] * wq_eff[c,h]      with wq_eff[c,h] = sum_d Wk[c,h*HD+d] q[h*HD+d]
#   w = exp(logits)          (inputs are bounded; softmax needs no max-subtraction)
#   z[h] = sum_n w[h,n]
#   s[h,c] = sum_n w[h,n] x[b,n,c]                  (attn-weighted token sum)
#   o[h,d] = (1/z[h]) sum_c s[h,c] Wv[c,h*HD+d]
#   out = o.flatten() @ Wp + bp
#
# This avoids materializing K and V entirely: the heavy work is two
# [N,C]-sized streaming contractions (logits and s) per batch instead of two
# [N,C]x[C,CR] projections — 16x fewer FLOPs.
#
# Sharding: data-parallel over batch. 8 cores x 4 batches each; weights
# replicated; no collectives. Per-core x shard is streamed in groups of 512
# tokens, cast fp32->bf16 in the DMA (SWDGE cast), transposed on the PE
# (needed because the logits contraction is over c, which must live on
# partitions), and consumed twice (logits from x^T, s-accum from x natural).
#
# The q / wq_eff prologue for ALL batches is computed once at kernel start
# from a tiny dedicated DMA of the class tokens (x[:,0,:]), so the steady
# state pipeline has no per-batch serialization.

import numpy as np
from contextlib import ExitStack

import concourse.bass as bass
import concourse.mybir as mybir
import concourse.tile as tile
from concourse import bacc
from concourse.masks import make_identity

F32 = mybir.dt.float32
BF16 = mybir.dt.bfloat16

B, N, C = 32, 4096, 1024
H, HD = 16, 16
CR = 256
SCALE = HD ** -0.5
NCORES = 8
BS = B // NCORES          # batches per core
GTOK = 512                # tokens per group
BLK = 128                 # tokens per block (partition tile)
NBLK = GTOK // BLK        # 4 blocks per group
NCB = C // 128            # 8 c-blocks


def emit(tc, x_d, cls_d, wq_d, wk_d, wv_d, wp_d, bp_d, dmask_d, sel_d, hsel_d, out_d, bs, n):
    nc = tc.nc
    ngroups = n // GTOK
    with ExitStack() as ctx:
        const = ctx.enter_context(tc.tile_pool(name="const", bufs=1))
        px = ctx.enter_context(tc.tile_pool(name="px", bufs=13))
        pxt = ctx.enter_context(tc.tile_pool(name="pxt", bufs=6))
        pw = ctx.enter_context(tc.tile_pool(name="pw", bufs=6))
        pb = ctx.enter_context(tc.tile_pool(name="pb", bufs=2))
        ps_xt = ctx.enter_context(tc.tile_pool(name="ps_xt", bufs=2, space="PSUM"))
        ps_lg = ctx.enter_context(tc.tile_pool(name="ps_lg", bufs=2, space="PSUM"))
        ps_sm = ctx.enter_context(tc.tile_pool(name="ps_sm", bufs=2, space="PSUM"))
        ps_s = ctx.enter_context(tc.tile_pool(name="ps_s", bufs=1, space="PSUM"))

        # ---- constants / weights ----
        ident = const.tile([128, 128], BF16)
        make_identity(nc, ident[:])

        cls_sb = const.tile([BS, C], BF16)
        nc.gpsimd.dma_start(out=cls_sb[:], in_=cls_d)
        wq_sb = const.tile([128, NCB, CR], BF16)     # Wq[c,r] c-blocked, bf16
        nc.gpsimd.dma_start(out=wq_sb[:], in_=wq_d.rearrange("(j p) r -> p j r", p=128))
        wk_sb = const.tile([128, NCB, CR], F32)      # Wk[c,r] fp32 (for wq_eff)
        nc.sync.dma_start(out=wk_sb[:], in_=wk_d.rearrange("(j p) r -> p j r", p=128))
        sel_sb = const.tile([BS, BS, 128], BF16)     # sel[p,b,i] = (p == b)
        nc.gpsimd.dma_start(out=sel_sb[:], in_=sel_d)
        # wv/wp are only needed by the first epilogue (~slot 10); their DMAs
        # are issued from inside the main loop so the first x-groups aren't
        # stuck behind them in the SWDGE queue.
        wv_sb = const.tile([128, NCB, CR], BF16)     # Wv[c,r]
        wp_sb = const.tile([128, 2, C], BF16)        # Wp[r,c] r-blocked
        bp_sb = const.tile([1, C], F32)
        nc.sync.dma_start(out=bp_sb[:], in_=bp_d.rearrange("(u c) -> u c", u=1))
        dmask_sb = const.tile([128, 2, H], F32)   # dmask[p,half,h] = (h == 8*half + p//16)
        nc.sync.dma_start(out=dmask_sb[:], in_=dmask_d)
        hsel_sb = const.tile([128, H], F32)       # hsel[p,h] = (p % 32 == h)
        nc.sync.dma_start(out=hsel_sb[:], in_=hsel_d)

        def emit_late_weights():
            nc.gpsimd.dma_start(
                out=wv_sb[:], in_=wv_d.rearrange("(j p) r -> p j r", p=128)
            )
            nc.gpsimd.dma_start(
                out=wp_sb[:], in_=wp_d.rearrange("(j p) c -> p j c", p=128)
            )

        bstate = {}   # per-batch: s_ps, zg_all
        wq_effs = {}  # per-batch: wq_eff_bf (computed once at start)
        gstate = {}   # per-(b,g): xg, xt
        wstate = {}   # per-(b,g): wT (exp output, consumed by C2)

        def emit_PRO_ALL():
            """q + wq_eff for ALL batches, from the dedicated cls DMA."""
            clsT_ps = ps_sm.tile([128, NCB, BS], BF16, tag="sm")
            for j in range(NCB):
                nc.tensor.transpose(
                    clsT_ps[:, j, :], cls_sb[0:BS, j * 128 : (j + 1) * 128],
                    ident[:BS, :BS],
                )
            clsT_sb = pb.tile([128, NCB, BS], BF16, tag="clsT", bufs=1)
            nc.vector.tensor_copy(clsT_sb[:], clsT_ps[:])
            q_ps = ps_sm.tile([BS, CR], F32, tag="sm")
            for j in range(NCB):
                nc.tensor.matmul(
                    q_ps[:], clsT_sb[:, j, :], wq_sb[:, j, :],
                    start=(j == 0), stop=(j == NCB - 1),
                )
            qs_bf = pb.tile([BS, CR], BF16, tag="qs", bufs=1)
            nc.scalar.mul(qs_bf[:], q_ps[:], SCALE)
            for b in range(bs):
                rep_ps = ps_sm.tile([128, CR], F32, tag="sm")
                nc.tensor.matmul(rep_ps[:], sel_sb[0:BS, b, :], qs_bf[0:BS, :])
                qs_rep = pb.tile([128, CR], F32, tag="qs_rep")
                nc.vector.tensor_copy(qs_rep[:], rep_ps[:])
                wq_eff = pb.tile([128, NCB, H], F32, tag="wq_eff")
                tmp = pb.tile([128, CR], F32, tag="tmp")
                for j in range(NCB):
                    nc.vector.tensor_mul(tmp[:], wk_sb[:, j, :], qs_rep[:])
                    nc.vector.reduce_sum(
                        wq_eff[:, j, :],
                        tmp.rearrange("p (h d) -> p h d", h=H),
                        axis=mybir.AxisListType.X,
                    )
                wq_eff_bf = pb.tile([128, NCB, H], BF16, tag="wq_eff_bf", bufs=BS)
                nc.vector.tensor_copy(wq_eff_bf[:], wq_eff[:])
                wq_effs[b] = wq_eff_bf

        def emit_BSTART(b):
            # s accumulates as two independent chains on PE col-strips q2/q3:
            # rows 64:80 hold s[:, 0:512], rows 96:112 hold s[:, 512:1024].
            s_ps = ps_s.tile([128, C], F32, tag="s")
            zg_all = pb.tile([128, ngroups], F32, tag="zg")
            nc.vector.memset(zg_all[:], 0.0)
            bstate[b] = (s_ps, zg_all)

        def emit_T(b, g):
            """Load + cast one 512-token group, transpose to xT via PE."""
            xg = px.tile([128, NBLK, C], BF16, tag="xg")
            # Token n' = t*128+p of this group holds DRAM token g*GTOK + 4p + t:
            # each partition reads 4 consecutive rows = 16 KB contiguous DRAM
            # per descriptor (4x fewer descriptors; attention is permutation-
            # invariant over tokens so any consistent order works).
            xsrc = x_d[b, g * GTOK : (g + 1) * GTOK, :].rearrange(
                "(p t) c -> p t c", t=NBLK
            )
            # Two half-transfers so the first blocks' transposes can start
            # after 1 MB instead of 2 MB (subtile deps track the halves).
            hb = NBLK // 2
            nc.gpsimd.dma_start(out=xg[:, 0:hb, :], in_=xsrc[:, 0:hb, :])
            nc.gpsimd.dma_start(out=xg[:, hb:NBLK, :], in_=xsrc[:, hb:NBLK, :])
            xt = pxt.tile([128, NCB, GTOK], BF16, tag="xt")
            for blk in range(NBLK):
                for jh in range(2):          # two 1-bank psum tiles per block
                    xt_ps = ps_xt.tile([128, 4, 128], BF16, tag="xt_ps")
                    for jj in range(4):
                        j = jh * 4 + jj
                        nc.tensor.transpose(
                            xt_ps[:, jj, :],
                            xg[:, blk, j * 128 : (j + 1) * 128], ident[:],
                        )
                    dst = xt[:, jh * 4 : (jh + 1) * 4, blk * BLK : (blk + 1) * BLK]
                    if blk % 2 == 0 or jh == 0:
                        nc.vector.tensor_copy(dst, xt_ps[:])
                    else:
                        nc.scalar.copy(dst, xt_ps[:])
            gstate[(b, g)] = (xg, xt)

        lg4 = {}  # rolling 2-strip logits bank, keyed by (b, g // 2)

        def gen_C1(b, g):
            """Logits MM thunks (col-strip sg = g%2) + trailing exp thunk."""
            _, xt = gstate[(b, g)]
            s_ps, zg_all = bstate[b]
            sg = g % 2
            if sg == 0:
                lg4[(b, g // 2)] = ps_lg.tile(
                    [128, GTOK], F32, tag="lg", name="lg4_ps"
                )
            lg_ps = lg4[(b, g // 2)]
            p0 = 32 * sg
            mms = []
            for j in range(NCB):
                mms.append(lambda j=j: nc.tensor.matmul(
                    lg_ps[p0 : p0 + 16, :], wq_effs[b][:, j, :], xt[:, j, :],
                    start=(j == 0), stop=(j == NCB - 1),
                    tile_position=(0, p0),
                ))

            def fin():
                wT = pw.tile([16, GTOK], BF16, tag="wT")
                nc.scalar.activation(
                    wT[:], lg_ps[p0 : p0 + 16, :],
                    mybir.ActivationFunctionType.Exp,
                    accum_out=zg_all[p0 : p0 + 16, g : g + 1],
                )
                wstate[(b, g)] = wT

            return mms, fin

        def emit_C2_wT(b, g):
            """w transposes into [n, h] layout for the s chains."""
            wT = wstate.pop((b, g))
            w_sb = pw.tile([128, NBLK, H], BF16, tag="w_sb")
            for blk in range(NBLK):
                w_ps = ps_sm.tile([128, H], BF16, tag="sm")
                nc.tensor.transpose(
                    w_ps[:], wT[:, blk * BLK : (blk + 1) * BLK], ident[:16, :16],
                )
                nc.vector.tensor_copy(w_sb[:, blk, :], w_ps[:])
            return w_sb

        def gen_C2(b, g, w_sb):
            """Two independent s-accumulation chains (strips q2 / q3)."""
            xg, _ = gstate.pop((b, g))
            s_ps, zg_all = bstate[b]
            chains = ([], [])
            for blk in range(NBLK):
                first = g == 0 and blk == 0
                last = g == ngroups - 1 and blk == NBLK - 1
                for half in range(2):
                    p0 = 64 + 32 * half
                    chains[half].append(lambda blk=blk, half=half, p0=p0, first=first, last=last: nc.tensor.matmul(
                        s_ps[p0 : p0 + 16, half * 512 : (half + 1) * 512],
                        w_sb[:, blk, :],
                        xg[:, blk, half * 512 : (half + 1) * 512],
                        start=first, stop=last,
                        tile_position=(0, p0),
                    ))
            return chains

        def emit_E(b):
            """Normalize s, project through Wv (block-diagonal), then Wp + bias."""
            s_ps, zg_all = bstate.pop(b)
            # z lives scattered at partition rows 32*sg + h; fold the strips
            # with tiny PE matmuls against the hsel selection matrix, placing
            # the result at rows 64:80 / 96:112 to match the s strips.
            zrow = pb.tile([128, 1], F32, tag="zrow")
            nc.vector.reduce_sum(zrow[:], zg_all[:], axis=mybir.AxisListType.X)
            z_ps4 = ps_sm.tile([128, 1], F32, tag="sm")
            nc.tensor.matmul(
                z_ps4[64:80, :], hsel_sb[:], zrow[:], tile_position=(0, 64)
            )
            nc.tensor.matmul(
                z_ps4[96:112, :], hsel_sb[:], zrow[:], tile_position=(0, 96)
            )
            rz4 = pb.tile([128, 1], F32, tag="rz")
            nc.vector.reciprocal(rz4[64:80, :], z_ps4[64:80, :])
            nc.vector.reciprocal(rz4[96:112, :], z_ps4[96:112, :])
            sbar4 = pb.tile([128, 512], BF16, tag="sbar")
            nc.vector.tensor_scalar_mul(
                sbar4[64:80, :], s_ps[64:80, 0:512], rz4[64:80, :]
            )
            nc.vector.tensor_scalar_mul(
                sbar4[96:112, :], s_ps[96:112, 512:1024], rz4[96:112, :]
            )
            stT = pb.tile([128, NCB, H], BF16, tag="stT")
            st_pss = []
            for j in range(NCB):
                if j < 4:
                    src = sbar4[64:80, j * 128 : (j + 1) * 128]
                    idn = ident[64:80, 64:80]
                    tp = (64, 0)
                else:
                    src = sbar4[96:112, (j - 4) * 128 : (j - 3) * 128]
                    idn = ident[96:112, 96:112]
                    tp = (96, 0)
                st_ps = ps_sm.tile([128, H], BF16, tag="sm")
                nc.tensor.transpose(st_ps[:], src, idn, tile_position=tp)
                st_pss.append(st_ps)
            for j in range(NCB):
                nc.vector.tensor_copy(stT[:, j, :], st_pss[j][:])
            # o_fullT[cr, h] = sum_c Wv[c, cr] * sbar[h, c]; keep only h == cr//HD
            o_flatT_f = pb.tile([128, 2], F32, tag="o_flatT_f")
            o_flatT = pb.tile([128, 2], BF16, tag="o_flatT")
            for half in range(2):
                of_ps = ps_sm.tile([128, H], F32, tag="sm")
                for j in range(NCB):
                    nc.tensor.matmul(
                        of_ps[:], wv_sb[:, j, half * 128 : (half + 1) * 128],
                        stT[:, j, :],
                        start=(j == 0), stop=(j == NCB - 1),
                    )
                om = pb.tile([128, H], F32, tag="om")
                nc.vector.tensor_mul(om[:], of_ps[:], dmask_sb[:, half, :])
                nc.vector.reduce_sum(
                    o_flatT_f[:, half : half + 1], om[:], axis=mybir.AxisListType.X
                )
            nc.vector.tensor_copy(o_flatT[:], o_flatT_f[:])
            # out = o_flat @ Wp + bp
            out_sb = pb.tile([1, C], F32, tag="out_sb")
            for half in range(2):
                op_ps = ps_sm.tile([1, 512], F32, tag="sm")
                for j in range(2):
                    nc.tensor.matmul(
                        op_ps[:], o_flatT[:, j : j + 1],
                        wp_sb[:, j, half * 512 : (half + 1) * 512],
                        start=(j == 0), stop=(j == 1),
                    )
                nc.vector.tensor_add(
                    out_sb[:, half * 512 : (half + 1) * 512], op_ps[:],
                    bp_sb[:, half * 512 : (half + 1) * 512],
                )
            nc.sync.dma_start(out=out_d[b : b + 1, :], in_=out_sb[:])

        # ---- software-pipelined emission ----
        # slot k: T(k), E (2 slots after last C2 of a batch), then the C2(k-2)
        # s-chains and C1(k-1) logits chain INTERLEAVED at matmul granularity
        # so the PE streams them concurrently on disjoint column strips.
        from collections import deque
        emit_PRO_ALL()
        items = [(b, g) for b in range(bs) for g in range(ngroups)]
        q_c1 = deque()   # waiting to be logits'd (2-slot lag)
        q_c2 = deque()   # waiting for w+s (1 more slot after C1)
        pend_epi = None
        def flush_slot():
            nonlocal pend_epi
            if pend_epi is not None:
                emit_E(pend_epi)
                pend_epi = None
            ca = cb = ()
            c1_mms, c1_fin = (), None
            if q_c2:
                it2 = q_c2.popleft()
                w_sb = emit_C2_wT(*it2)
                ca, cb = gen_C2(*it2, w_sb)
                if it2[1] == ngroups - 1:
                    pend_epi = it2[0]
            if len(q_c1) >= 2:
                it1 = q_c1.popleft()
                c1_mms, c1_fin = gen_C1(*it1)
                q_c2.append(it1)
            for i in range(max(len(ca), len(cb), len(c1_mms))):
                if i < len(ca):
                    ca[i]()
                if i < len(cb):
                    cb[i]()
                if i < len(c1_mms):
                    c1_mms[i]()
            if c1_fin is not None:
                c1_fin()
        for idx, it in enumerate(items):
            if it[1] == 0:
                emit_BSTART(it[0])
            emit_T(*it)
            if idx == 5:
                emit_late_weights()
            flush_slot()
            q_c1.append(it)
        for _ in range(5):
            if len(q_c1) == 1:
                c1_mms, c1_fin = gen_C1(*q_c1[0])
                q_c2.append(q_c1.popleft())
                for m in c1_mms:
                    m()
                c1_fin()
            flush_slot()


def make_dmask():
    dm = np.zeros((128, 2, H), dtype=np.float32)
    for p in range(128):
        for half in range(2):
            dm[p, half, 8 * half + p // 16] = 1.0
    return dm


def make_sel():
    s = np.zeros((BS, BS, 128), dtype=np.float32)
    for b in range(BS):
        s[b, b, :] = 1.0
    return s


def make_hsel():
    s = np.zeros((128, H), dtype=np.float32)
    for p in range(128):
        if p % 32 < H:
            s[p, p % 32] = 1.0
    return s


def build_bass(bs=BS, n=N):
    nc = bacc.Bacc("TRN2", target_bir_lowering=False, debug=False, num_devices=NCORES)
    x_d = nc.dram_tensor("x", [bs, n, C], F32, kind="ExternalInput").ap()
    cls_d = nc.dram_tensor("cls", [bs, C], F32, kind="ExternalInput").ap()
    wq_d = nc.dram_tensor("Wq", [C, CR], F32, kind="ExternalInput").ap()
    wk_d = nc.dram_tensor("Wk", [C, CR], F32, kind="ExternalInput").ap()
    wv_d = nc.dram_tensor("Wv", [C, CR], F32, kind="ExternalInput").ap()
    wp_d = nc.dram_tensor("Wp", [CR, C], F32, kind="ExternalInput").ap()
    bp_d = nc.dram_tensor("bp", [C], F32, kind="ExternalInput").ap()
    dmask_d = nc.dram_tensor("dmask", [128, 2, H], F32, kind="ExternalInput").ap()
    sel_d = nc.dram_tensor("sel", [BS, BS, 128], F32, kind="ExternalInput").ap()
    hsel_d = nc.dram_tensor("hsel", [128, H], F32, kind="ExternalInput").ap()
    out_d = nc.dram_tensor("out", [bs, C], F32, kind="ExternalOutput").ap()
    with tile.TileContext(nc) as tc:
        emit(tc, x_d, cls_d, wq_d, wk_d, wv_d, wp_d, bp_d, dmask_d, sel_d, hsel_d, out_d, bs, n)
    nc.compile()
    return nc


def make_in_maps(x, wq, wk, wv, wp, bp):
    dmask = make_dmask()
    sel = make_sel()
    hsel = make_hsel()
    return [
        {
            "x": np.ascontiguousarray(x[c * BS : (c + 1) * BS]),
            "cls": np.ascontiguousarray(x[c * BS : (c + 1) * BS, 0, :]),
            "Wq": wq, "Wk": wk, "Wv": wv, "Wp": wp, "bp": bp,
            "dmask": dmask, "sel": sel, "hsel": hsel,
        }
        for c in range(NCORES)
    ]


def kernel(**inputs):
    from concourse.bass_utils import run_bass_kernel_spmd

    x = np.ascontiguousarray(np.asarray(inputs["x"], dtype=np.float32))
    wq = np.ascontiguousarray(np.asarray(inputs["Wq"], dtype=np.float32))
    wk = np.ascontiguousarray(np.asarray(inputs["Wk"], dtype=np.float32))
    wv = np.ascontiguousarray(np.asarray(inputs["Wv"], dtype=np.float32))
    wp = np.ascontiguousarray(np.asarray(inputs["Wp"], dtype=np.float32))
    bp = np.ascontiguousarray(np.asarray(inputs["bp"], dtype=np.float32))

    nc = build_bass()
    in_maps = make_in_maps(x, wq, wk, wv, wp, bp)
    res = run_bass_kernel_spmd(nc, in_maps, core_ids=list(range(NCORES)))
    out = np.concatenate([r["out"] for r in res.results], axis=0)  # [B, C]
    return out.reshape(B, 1, C).astype(np.float32)


# revision 33
# speedup vs baseline: 1.0721x; 1.0721x over previous
# Trainium2 Bass kernel for ClassAttn (single class-token query attention).
#
# Math (per batch b):
#   q   = x[b,0] @ Wq * scale                       [CR]
#   logits[h,n] = sum_d q[h,d] * (x[b] @ Wk)[n,h,d]
#               = sum_c x[b,n,c] * wq_eff[c,h]      with wq_eff[c,h] = sum_d Wk[c,h*HD+d] q[h*HD+d]
#   w = exp(logits)          (inputs are bounded; softmax needs no max-subtraction)
#   z[h] = sum_n w[h,n]
#   s[h,c] = sum_n w[h,n] x[b,n,c]                  (attn-weighted token sum)
#   o[h,d] = (1/z[h]) sum_c s[h,c] Wv[c,h*HD+d]
#   out = o.flatten() @ Wp + bp
#
# This avoids materializing K and V entirely: the heavy work is two
# [N,C]-sized streaming contractions (logits and s) per batch instead of two
# [N,C]x[C,CR] projections — 16x fewer FLOPs.
#
# Sharding: data-parallel over batch. 8 cores x 4 batches each; weights
# replicated; no collectives. Per-core x shard is streamed in groups of 512
# tokens, cast fp32->bf16 in the DMA (SWDGE cast), transposed on the PE
# (needed because the logits contraction is over c, which must live on
# partitions), and consumed twice (logits from x^T, s-accum from x natural).
#
# The q / wq_eff prologue for ALL batches is computed once at kernel start
# from a tiny dedicated DMA of the class tokens (x[:,0,:]), so the steady
# state pipeline has no per-batch serialization.

import numpy as np
from contextlib import ExitStack

import concourse.bass as bass
import concourse.mybir as mybir
import concourse.tile as tile
from concourse import bacc
from concourse.masks import make_identity

F32 = mybir.dt.float32
BF16 = mybir.dt.bfloat16

B, N, C = 32, 4096, 1024
H, HD = 16, 16
CR = 256
SCALE = HD ** -0.5
NCORES = 8
BS = B // NCORES          # batches per core
GTOK = 512                # tokens per group
BLK = 128                 # tokens per block (partition tile)
NBLK = GTOK // BLK        # 4 blocks per group
NCB = C // 128            # 8 c-blocks


def emit(tc, x_d, cls_d, wq_d, wk_d, wv_d, wp_d, bp_d, dmask_d, sel_d, hsel_d, out_d, bs, n):
    nc = tc.nc
    ngroups = n // GTOK
    with ExitStack() as ctx:
        const = ctx.enter_context(tc.tile_pool(name="const", bufs=1))
        px = ctx.enter_context(tc.tile_pool(name="px", bufs=13))
        pxt = ctx.enter_context(tc.tile_pool(name="pxt", bufs=6))
        pw = ctx.enter_context(tc.tile_pool(name="pw", bufs=6))
        pb = ctx.enter_context(tc.tile_pool(name="pb", bufs=2))
        ps_xt = ctx.enter_context(tc.tile_pool(name="ps_xt", bufs=3, space="PSUM"))
        ps_lg = ctx.enter_context(tc.tile_pool(name="ps_lg", bufs=2, space="PSUM"))
        ps_sm = ctx.enter_context(tc.tile_pool(name="ps_sm", bufs=1, space="PSUM"))
        ps_s = ctx.enter_context(tc.tile_pool(name="ps_s", bufs=1, space="PSUM"))

        # ---- constants / weights ----
        ident = const.tile([128, 128], BF16)
        make_identity(nc, ident[:])

        cls_sb = const.tile([BS, C], BF16)
        nc.gpsimd.dma_start(out=cls_sb[:], in_=cls_d)
        wq_sb = const.tile([128, NCB, CR], BF16)     # Wq[c,r] c-blocked, bf16
        nc.gpsimd.dma_start(out=wq_sb[:], in_=wq_d.rearrange("(j p) r -> p j r", p=128))
        wk_sb = const.tile([128, NCB, CR], F32)      # Wk[c,r] fp32 (for wq_eff)
        nc.sync.dma_start(out=wk_sb[:], in_=wk_d.rearrange("(j p) r -> p j r", p=128))
        sel_sb = const.tile([BS, BS, 128], BF16)     # sel[p,b,i] = (p == b)
        nc.gpsimd.dma_start(out=sel_sb[:], in_=sel_d)
        # wv/wp are only needed by the first epilogue (~slot 10); their DMAs
        # are issued from inside the main loop so the first x-groups aren't
        # stuck behind them in the SWDGE queue.
        wv_sb = const.tile([128, NCB, CR], BF16)     # Wv[c,r]
        wp_sb = const.tile([128, 2, C], BF16)        # Wp[r,c] r-blocked
        bp_sb = const.tile([1, C], F32)
        nc.sync.dma_start(out=bp_sb[:], in_=bp_d.rearrange("(u c) -> u c", u=1))
        dmask_sb = const.tile([128, 2, H], F32)   # dmask[p,half,h] = (h == 8*half + p//16)
        nc.sync.dma_start(out=dmask_sb[:], in_=dmask_d)
        hsel_sb = const.tile([128, H], F32)       # hsel[p,h] = (p % 32 == h)
        nc.sync.dma_start(out=hsel_sb[:], in_=hsel_d)

        def emit_late_weights():
            nc.gpsimd.dma_start(
                out=wv_sb[:], in_=wv_d.rearrange("(j p) r -> p j r", p=128)
            )
            nc.gpsimd.dma_start(
                out=wp_sb[:], in_=wp_d.rearrange("(j p) c -> p j c", p=128)
            )

        bstate = {}   # per-batch: s_ps, zg_all
        wq_effs = {}  # per-batch: wq_eff_bf (computed once at start)
        gstate = {}   # per-(b,g): xg, xt
        wstate = {}   # per-(b,g): wT (exp output, consumed by C2)

        def emit_PRO_ALL():
            """q + wq_eff for ALL batches, from the dedicated cls DMA."""
            clsT_ps = ps_sm.tile([128, NCB, BS], BF16, tag="sm")
            for j in range(NCB):
                nc.tensor.transpose(
                    clsT_ps[:, j, :], cls_sb[0:BS, j * 128 : (j + 1) * 128],
                    ident[:BS, :BS],
                )
            clsT_sb = pb.tile([128, NCB, BS], BF16, tag="clsT", bufs=1)
            nc.vector.tensor_copy(clsT_sb[:], clsT_ps[:])
            q_ps = ps_sm.tile([BS, CR], F32, tag="sm")
            for j in range(NCB):
                nc.tensor.matmul(
                    q_ps[:], clsT_sb[:, j, :], wq_sb[:, j, :],
                    start=(j == 0), stop=(j == NCB - 1),
                )
            qs_bf = pb.tile([BS, CR], BF16, tag="qs", bufs=1)
            nc.scalar.mul(qs_bf[:], q_ps[:], SCALE)
            for b in range(bs):
                rep_ps = ps_sm.tile([128, CR], F32, tag="sm")
                nc.tensor.matmul(rep_ps[:], sel_sb[0:BS, b, :], qs_bf[0:BS, :])
                qs_rep = pb.tile([128, CR], F32, tag="qs_rep")
                nc.vector.tensor_copy(qs_rep[:], rep_ps[:])
                wq_eff = pb.tile([128, NCB, H], F32, tag="wq_eff")
                tmp = pb.tile([128, CR], F32, tag="tmp")
                for j in range(NCB):
                    nc.vector.tensor_mul(tmp[:], wk_sb[:, j, :], qs_rep[:])
                    nc.vector.reduce_sum(
                        wq_eff[:, j, :],
                        tmp.rearrange("p (h d) -> p h d", h=H),
                        axis=mybir.AxisListType.X,
                    )
                wq_eff_bf = pb.tile([128, NCB, H], BF16, tag="wq_eff_bf", bufs=BS)
                nc.vector.tensor_copy(wq_eff_bf[:], wq_eff[:])
                wq_effs[b] = wq_eff_bf

        def emit_BSTART(b):
            # s accumulates as two independent chains on PE col-strips q2/q3:
            # rows 64:80 hold s[:, 0:512], rows 96:112 hold s[:, 512:1024].
            s_ps = ps_s.tile([128, C], F32, tag="s")
            zg_all = pb.tile([128, ngroups], F32, tag="zg")
            nc.vector.memset(zg_all[:], 0.0)
            bstate[b] = (s_ps, zg_all)

        def emit_T(b, g):
            """Load + cast one 512-token group, transpose to xT via PE."""
            xg = px.tile([128, NBLK, C], BF16, tag="xg")
            # Token n' = t*128+p of this group holds DRAM token g*GTOK + 4p + t:
            # each partition reads 4 consecutive rows = 16 KB contiguous DRAM
            # per descriptor (4x fewer descriptors; attention is permutation-
            # invariant over tokens so any consistent order works).
            xsrc = x_d[b, g * GTOK : (g + 1) * GTOK, :].rearrange(
                "(p t) c -> p t c", t=NBLK
            )
            # Two half-transfers so the first blocks' transposes can start
            # after 1 MB instead of 2 MB (subtile deps track the halves).
            hb = NBLK // 2
            nc.gpsimd.dma_start(out=xg[:, 0:hb, :], in_=xsrc[:, 0:hb, :])
            nc.gpsimd.dma_start(out=xg[:, hb:NBLK, :], in_=xsrc[:, hb:NBLK, :])
            xt = pxt.tile([128, NCB, GTOK], BF16, tag="xt")
            for blk in range(NBLK):
                for jh in range(2):          # two 1-bank psum tiles per block
                    xt_ps = ps_xt.tile([128, 4, 128], BF16, tag="xt_ps")
                    for jj in range(4):
                        j = jh * 4 + jj
                        nc.tensor.transpose(
                            xt_ps[:, jj, :],
                            xg[:, blk, j * 128 : (j + 1) * 128], ident[:],
                        )
                    dst = xt[:, jh * 4 : (jh + 1) * 4, blk * BLK : (blk + 1) * BLK]
                    if blk % 2 == 0 or jh == 0:
                        nc.vector.tensor_copy(dst, xt_ps[:])
                    else:
                        nc.scalar.copy(dst, xt_ps[:])
            gstate[(b, g)] = (xg, xt)

        lg4 = {}   # rolling 2-strip logits bank, keyed by (b, g // 2)
        wT4 = {}   # exp outputs for a group-pair, strips packed in partitions

        def gen_C1(b, g):
            """Logits MM thunks (col-strip sg = g%2) + trailing exp thunk."""
            _, xt = gstate[(b, g)]
            s_ps, zg_all = bstate[b]
            sg = g % 2
            if sg == 0:
                lg4[(b, g // 2)] = ps_lg.tile(
                    [128, GTOK], F32, tag="lg", name="lg4_ps"
                )
                wT4[(b, g // 2)] = pw.tile(
                    [128, GTOK], BF16, tag="wT4", name="wT4_sb", bufs=2
                )
            lg_ps = lg4[(b, g // 2)]
            wT_sb = wT4[(b, g // 2)]
            p0 = 32 * sg
            mms = []
            for j in range(NCB):
                mms.append(lambda j=j: nc.tensor.matmul(
                    lg_ps[p0 : p0 + 16, :], wq_effs[b][:, j, :], xt[:, j, :],
                    start=(j == 0), stop=(j == NCB - 1),
                    tile_position=(0, p0),
                ))

            def fin():
                nc.scalar.activation(
                    wT_sb[p0 : p0 + 16, :], lg_ps[p0 : p0 + 16, :],
                    mybir.ActivationFunctionType.Exp,
                    accum_out=zg_all[p0 : p0 + 16, g : g + 1],
                )

            return mms, fin

        w4Ts = {}  # transposed w for a group-pair, keyed (b, q)

        def emit_C2_wT(b, q):
            """Batched w transposes for group-pair q: both strips in one
            [128,128] transpose per token block."""
            wT_sb = wT4.pop((b, q))
            w4T_sb = pw.tile([128, NBLK, BLK], BF16, tag="w4T", bufs=2)
            for blk in range(NBLK):
                w_ps = ps_sm.tile([128, BLK], BF16, tag="sm", name="w_ps")
                nc.tensor.transpose(
                    w_ps[:], wT_sb[:, blk * BLK : (blk + 1) * BLK], ident[:],
                )
                nc.vector.tensor_copy(w4T_sb[:, blk, :], w_ps[:])
            w4Ts[(b, q)] = w4T_sb

        def gen_C2(b, g):
            """Two independent s-accumulation chains (strips q2 / q3)."""
            xg, _ = gstate.pop((b, g))
            s_ps, zg_all = bstate[b]
            w4T_sb = w4Ts[(b, g // 2)]
            if g % 2 == 1:
                del w4Ts[(b, g // 2)]
            c0 = 32 * (g % 2)
            chains = ([], [])
            for blk in range(NBLK):
                first = g == 0 and blk == 0
                last = g == ngroups - 1 and blk == NBLK - 1
                for half in range(2):
                    p0 = 64 + 32 * half
                    chains[half].append(lambda blk=blk, half=half, p0=p0, first=first, last=last: nc.tensor.matmul(
                        s_ps[p0 : p0 + 16, half * 512 : (half + 1) * 512],
                        w4T_sb[:, blk, c0 : c0 + 16],
                        xg[:, blk, half * 512 : (half + 1) * 512],
                        start=first, stop=last,
                        tile_position=(0, p0),
                    ))
            return chains

        def emit_E(b):
            """Normalize s, project through Wv (block-diagonal), then Wp + bias."""
            s_ps, zg_all = bstate.pop(b)
            # z lives scattered at partition rows 32*sg + h; fold the strips
            # with tiny PE matmuls against the hsel selection matrix, placing
            # the result at rows 64:80 / 96:112 to match the s strips.
            zrow = pb.tile([128, 1], F32, tag="zrow")
            nc.vector.reduce_sum(zrow[:], zg_all[:], axis=mybir.AxisListType.X)
            z_ps4 = ps_sm.tile([128, 1], F32, tag="sm")
            nc.tensor.matmul(
                z_ps4[64:80, :], hsel_sb[:], zrow[:], tile_position=(0, 64)
            )
            nc.tensor.matmul(
                z_ps4[96:112, :], hsel_sb[:], zrow[:], tile_position=(0, 96)
            )
            rz4 = pb.tile([128, 1], F32, tag="rz")
            nc.vector.reciprocal(rz4[64:80, :], z_ps4[64:80, :])
            nc.vector.reciprocal(rz4[96:112, :], z_ps4[96:112, :])
            sbar4 = pb.tile([128, 512], BF16, tag="sbar")
            nc.vector.tensor_scalar_mul(
                sbar4[64:80, :], s_ps[64:80, 0:512], rz4[64:80, :]
            )
            nc.vector.tensor_scalar_mul(
                sbar4[96:112, :], s_ps[96:112, 512:1024], rz4[96:112, :]
            )
            stT = pb.tile([128, NCB, H], BF16, tag="stT")
            for q in range(4):
                st_ps = ps_sm.tile([128, BLK], BF16, tag="sm", name="st_ps")
                nc.tensor.transpose(
                    st_ps[:], sbar4[:, q * 128 : (q + 1) * 128], ident[:]
                )
                nc.vector.tensor_copy(stT[:, q, :], st_ps[:, 64:80])
                nc.vector.tensor_copy(stT[:, q + 4, :], st_ps[:, 96:112])
            # o_fullT[cr, h] = sum_c Wv[c, cr] * sbar[h, c]; keep only h == cr//HD
            o_flatT_f = pb.tile([128, 2], F32, tag="o_flatT_f")
            o_flatT = pb.tile([128, 2], BF16, tag="o_flatT")
            for half in range(2):
                of_ps = ps_sm.tile([128, H], F32, tag="sm")
                for j in range(NCB):
                    nc.tensor.matmul(
                        of_ps[:], wv_sb[:, j, half * 128 : (half + 1) * 128],
                        stT[:, j, :],
                        start=(j == 0), stop=(j == NCB - 1),
                    )
                om = pb.tile([128, H], F32, tag="om")
                nc.vector.tensor_mul(om[:], of_ps[:], dmask_sb[:, half, :])
                nc.vector.reduce_sum(
                    o_flatT_f[:, half : half + 1], om[:], axis=mybir.AxisListType.X
                )
            nc.vector.tensor_copy(o_flatT[:], o_flatT_f[:])
            # out = o_flat @ Wp + bp
            out_sb = pb.tile([1, C], F32, tag="out_sb")
            for half in range(2):
                op_ps = ps_sm.tile([1, 512], F32, tag="sm")
                for j in range(2):
                    nc.tensor.matmul(
                        op_ps[:], o_flatT[:, j : j + 1],
                        wp_sb[:, j, half * 512 : (half + 1) * 512],
                        start=(j == 0), stop=(j == 1),
                    )
                nc.vector.tensor_add(
                    out_sb[:, half * 512 : (half + 1) * 512], op_ps[:],
                    bp_sb[:, half * 512 : (half + 1) * 512],
                )
            nc.sync.dma_start(out=out_d[b : b + 1, :], in_=out_sb[:])

        # ---- software-pipelined emission ----
        # slot k: T(k), E (2 slots after last C2 of a batch), then the C2(k-2)
        # s-chains and C1(k-1) logits chain INTERLEAVED at matmul granularity
        # so the PE streams them concurrently on disjoint column strips.
        from collections import deque
        emit_PRO_ALL()
        items = [(b, g) for b in range(bs) for g in range(ngroups)]
        q_c1 = deque()   # waiting to be logits'd (2-slot lag)
        q_c2 = deque()   # waiting for w+s (1 more slot after C1)
        pend_epi = None

        def flush_pre(min_c2=2):
            """Epilogue + batched w-transposes, emitted BEFORE the T band so
            their DVE copies hide under the transpose stream. C2 lags C1 by
            2 slots so a pair's odd-group exp is emitted before the pair's
            batched w-transpose consumes it."""
            nonlocal pend_epi
            if pend_epi is not None:
                emit_E(pend_epi)
                pend_epi = None
            it2 = q_c2.popleft() if len(q_c2) >= min_c2 else None
            if it2 is not None and it2[1] % 2 == 0:
                emit_C2_wT(it2[0], it2[1] // 2)
            return it2

        def flush_post(it2):
            nonlocal pend_epi
            ca = cb = ()
            c1_mms, c1_fin = (), None
            if it2 is not None:
                ca, cb = gen_C2(*it2)
                if it2[1] == ngroups - 1:
                    pend_epi = it2[0]
            if len(q_c1) >= 2:
                it1 = q_c1.popleft()
                c1_mms, c1_fin = gen_C1(*it1)
                q_c2.append(it1)
            for i in range(max(len(ca), len(cb), len(c1_mms))):
                if i < len(c1_mms):
                    c1_mms[i]()
                if i < len(ca):
                    ca[i]()
                if i < len(cb):
                    cb[i]()
            if c1_fin is not None:
                c1_fin()

        for idx, it in enumerate(items):
            if it[1] == 0:
                emit_BSTART(it[0])
            it2 = flush_pre()
            emit_T(*it)
            if idx == 5:
                emit_late_weights()
            flush_post(it2)
            q_c1.append(it)
        for _ in range(8):
            if len(q_c1) == 1 and len(q_c2) <= 1:
                c1_mms, c1_fin = gen_C1(*q_c1[0])
                q_c2.append(q_c1.popleft())
                for m in c1_mms:
                    m()
                c1_fin()
            it2 = flush_pre(min_c2=1)
            flush_post(it2)


def make_dmask():
    dm = np.zeros((128, 2, H), dtype=np.float32)
    for p in range(128):
        for half in range(2):
            dm[p, half, 8 * half + p // 16] = 1.0
    return dm


def make_sel():
    s = np.zeros((BS, BS, 128), dtype=np.float32)
    for b in range(BS):
        s[b, b, :] = 1.0
    return s


def make_hsel():
    s = np.zeros((128, H), dtype=np.float32)
    for p in range(128):
        if p % 32 < H:
            s[p, p % 32] = 1.0
    return s


def build_bass(bs=BS, n=N):
    nc = bacc.Bacc("TRN2", target_bir_lowering=False, debug=False, num_devices=NCORES)
    x_d = nc.dram_tensor("x", [bs, n, C], F32, kind="ExternalInput").ap()
    cls_d = nc.dram_tensor("cls", [bs, C], F32, kind="ExternalInput").ap()
    wq_d = nc.dram_tensor("Wq", [C, CR], F32, kind="ExternalInput").ap()
    wk_d = nc.dram_tensor("Wk", [C, CR], F32, kind="ExternalInput").ap()
    wv_d = nc.dram_tensor("Wv", [C, CR], F32, kind="ExternalInput").ap()
    wp_d = nc.dram_tensor("Wp", [CR, C], F32, kind="ExternalInput").ap()
    bp_d = nc.dram_tensor("bp", [C], F32, kind="ExternalInput").ap()
    dmask_d = nc.dram_tensor("dmask", [128, 2, H], F32, kind="ExternalInput").ap()
    sel_d = nc.dram_tensor("sel", [BS, BS, 128], F32, kind="ExternalInput").ap()
    hsel_d = nc.dram_tensor("hsel", [128, H], F32, kind="ExternalInput").ap()
    out_d = nc.dram_tensor("out", [bs, C], F32, kind="ExternalOutput").ap()
    with tile.TileContext(nc) as tc:
        emit(tc, x_d, cls_d, wq_d, wk_d, wv_d, wp_d, bp_d, dmask_d, sel_d, hsel_d, out_d, bs, n)
    nc.compile()
    return nc


def make_in_maps(x, wq, wk, wv, wp, bp):
    dmask = make_dmask()
    sel = make_sel()
    hsel = make_hsel()
    return [
        {
            "x": np.ascontiguousarray(x[c * BS : (c + 1) * BS]),
            "cls": np.ascontiguousarray(x[c * BS : (c + 1) * BS, 0, :]),
            "Wq": wq, "Wk": wk, "Wv": wv, "Wp": wp, "bp": bp,
            "dmask": dmask, "sel": sel, "hsel": hsel,
        }
        for c in range(NCORES)
    ]


def kernel(**inputs):
    from concourse.bass_utils import run_bass_kernel_spmd

    x = np.ascontiguousarray(np.asarray(inputs["x"], dtype=np.float32))
    wq = np.ascontiguousarray(np.asarray(inputs["Wq"], dtype=np.float32))
    wk = np.ascontiguousarray(np.asarray(inputs["Wk"], dtype=np.float32))
    wv = np.ascontiguousarray(np.asarray(inputs["Wv"], dtype=np.float32))
    wp = np.ascontiguousarray(np.asarray(inputs["Wp"], dtype=np.float32))
    bp = np.ascontiguousarray(np.asarray(inputs["bp"], dtype=np.float32))

    nc = build_bass()
    in_maps = make_in_maps(x, wq, wk, wv, wp, bp)
    res = run_bass_kernel_spmd(nc, in_maps, core_ids=list(range(NCORES)))
    out = np.concatenate([r["out"] for r in res.results], axis=0)  # [B, C]
    return out.reshape(B, 1, C).astype(np.float32)
